# revision 1
# baseline (speedup 1.0000x reference)
"""Trainium2 Bass kernel for nn_ComputeIdsLayer (sequential new-entity ID assignment).

Reference semantics (per batch element b):
  - used0 = set of ids appearing in enref_ids[b, :seq_len[b]]
  - scanning s = 0..S-1: if is_new[b,s] (logits[...,0] > 0), assign the smallest
    unused id, emit its one-hot, mark it used; else emit zeros.

Key reduction: assigned ids are consumed from the ascending sorted free-id list
F = sorted({0..127} \\ used0). The j-th new position gets F[j]; once j >= |F|,
argmax(~used) over an all-True mask returns 0, so every later new position gets
id 0. Therefore with
    k[b,s]   = exclusive-cumsum of is_new  (count of new positions before s)
    rank[b,n]= exclusive-cumsum over n of free0[b,n]   (rank of id n among free)
the output is a single equality compare per element:
    rank'[b,n] = rank[b,n] if free0[b,n] else -(n+10)      (distinct sentinels)
    r0[b]      = rank'[b,0]  (= 0 if id0 free else -10)
    k4[b,s]    = -2                     if not is_new
               = k[b,s]                 if k < nfree
               = r0[b]                  otherwise (overflow -> id 0)
    out[b,s,n] = (rank'[b,n] == k4[b,s])
All values are small integers, exact in fp32.

Layout: each core gets 32 batches; each sequence is split into 4 quarters of
512 so partitions = (b, q) use all 128 lanes. The used-id set is built as 8
16-bit limb bitmasks OR-reduced with a log-tree; cross-partition combines
(quarter carries for the cumsum, quarter-OR of limbs, rank broadcast) go
through tiny DRAM bounce buffers.

Sharding: pure data parallel over batch (256 -> 32 per core x 8 cores).
"""

import os
import sys

import numpy as np

for _p in ("/opt/trn_rl_repo",):
    if _p not in sys.path and os.path.isdir(_p):
        sys.path.insert(0, _p)

B_FULL = 256
N_CORES = 8
B = B_FULL // N_CORES  # 32 per core
S = 2048
N = 128  # id space
Q = 4  # sequence quarters
SQ = S // Q  # 512
P = B * Q  # 128 partitions
NBLK = 16  # output column blocks
BLK = SQ // NBLK  # 32 s-positions per block


def build_program():
    import concourse.bacc as bacc
    import concourse.mybir as mybir
    import concourse.tile as tile

    f32 = mybir.dt.float32
    i32 = mybir.dt.int32
    Alu = mybir.AluOpType

    nc = bacc.Bacc(
        "TRN2",
        target_bir_lowering=False,
        debug=False,
        enable_asserts=False,
        num_devices=N_CORES,
    )

    ids_d = nc.declare_dram_parameter("enref_ids", [B, S], i32, isOutput=False)
    len_d = nc.declare_dram_parameter("enref_seq_len", [B], i32, isOutput=False)
    log_d = nc.declare_dram_parameter("is_new_logits", [B, S, 2], f32, isOutput=False)
    out_d = nc.declare_dram_parameter("out", [B, S, N], f32, isOutput=True)

    with tile.TileContext(nc) as tc:
        with (
            tc.tile_pool(name="persist", bufs=1) as pp,
            tc.tile_pool(name="tmpq", bufs=4) as tmpp,
            tc.tile_pool(name="outp", bufs=int(os.environ.get("K_OBUFS", "4"))) as outp,
            tc.tile_pool(name="dram", bufs=1, space="DRAM") as dramp,
        ):
            # ---------------- loads (quarter layout) ----------------
            L4 = pp.tile([P, 1], i32, tag="L4")
            nc.sync.dma_start(
                out=L4[:],
                in_=len_d[:].unsqueeze(1).broadcast_to([B, Q]),
            )
            ids_q = pp.tile([P, SQ], i32, tag="ids_q")
            nc.sync.dma_start(
                out=ids_q[:], in_=ids_d[:].rearrange("b (q x) -> (b q) x", q=Q)
            )
            lg_q = pp.tile([P, 2 * SQ], f32, tag="lg_q")
            nc.sync.dma_start(
                out=lg_q[:], in_=log_d[:].rearrange("b (q x) c -> (b q) (x c)", q=Q)
            )

            # ---------------- iotas ----------------
            iota512 = pp.tile([P, SQ], i32, tag="iota512")
            nc.gpsimd.iota(iota512[:], pattern=[[1, SQ]], base=0, channel_multiplier=0)
            iotap = pp.tile([P, 1], i32, tag="iotap")
            nc.gpsimd.iota(iotap[:], pattern=[[0, 1]], base=0, channel_multiplier=1)
            iota16 = pp.tile([P, N], i32, tag="iota16")  # n & 15 per (l, j)
            nc.gpsimd.iota(iota16[:], pattern=[[0, 8], [1, 16]], base=0,
                           channel_multiplier=0)
            iota_n = pp.tile([P, N], i32, tag="iota_n")
            nc.gpsimd.iota(iota_n[:], pattern=[[1, N]], base=10, channel_multiplier=0)

            # ---------------- valid mask / is_new / cumsum ----------------
            # per-partition sequence offset: q*SQ where q = p & 3
            qcol_i = pp.tile([P, 1], i32, tag="qcol_i")
            nc.vector.tensor_single_scalar(
                out=qcol_i[:], in_=iotap[:], scalar=3, op=Alu.bitwise_and
            )
            qcol = pp.tile([P, 1], f32, tag="qcol")
            nc.vector.tensor_single_scalar(
                out=qcol[:], in_=qcol_i[:], scalar=float(SQ), op=Alu.mult
            )
            L4f = pp.tile([P, 1], f32, tag="L4f")
            nc.vector.tensor_copy(L4f[:], L4[:])
            Lqf = pp.tile([P, 1], f32, tag="Lqf")  # L - q*SQ
            nc.vector.tensor_sub(Lqf[:], L4f[:], qcol[:])
            valid = pp.tile([P, SQ], f32, tag="valid")
            nc.vector.tensor_scalar(
                out=valid[:], in0=iota512[:], scalar1=Lqf[:, 0:1], scalar2=None,
                op0=Alu.is_lt,
            )
            isnew = pp.tile([P, SQ], f32, tag="isnew")
            nc.vector.tensor_single_scalar(
                out=isnew[:], in_=lg_q[:, 0 : 2 * SQ : 2], scalar=0.0, op=Alu.is_gt
            )
            zerosq = pp.tile([P, SQ], f32, tag="zerosq")
            nc.vector.memset(zerosq[:], 0.0)
            kincl = pp.tile([P, SQ], f32, tag="kincl")
            nc.vector.tensor_tensor_scan(
                out=kincl[:], data0=zerosq[:], data1=isnew[:], initial=0.0,
                op0=Alu.add, op1=Alu.add,
            )

            # quarter carry: bounce quarter totals to DRAM, read back
            # replicated so every (b, q) partition sees all 4 totals, then
            # carry = sum of totals with q' < q, computed per partition.
            t_dram = dramp.tile([P], f32, tag="t_dram")
            nc.sync.dma_start(out=t_dram[:], in_=kincl[:, SQ - 1 : SQ])
            Tall = pp.tile([P, Q], f32, tag="Tall")
            nc.sync.dma_start(
                out=Tall[:],
                in_=t_dram[:].rearrange("(b q) -> b q", q=Q).unsqueeze(1)
                .broadcast_to([B, Q, Q]),
            )
            iota4 = pp.tile([P, Q], i32, tag="iota4")
            nc.gpsimd.iota(iota4[:], pattern=[[1, Q]], base=0, channel_multiplier=0)
            qf = pp.tile([P, 1], f32, tag="qf")
            nc.vector.tensor_copy(qf[:], qcol_i[:])
            qlt = pp.tile([P, Q], f32, tag="qlt")  # 1 where q' < q
            nc.vector.tensor_scalar(
                out=qlt[:], in0=iota4[:], scalar1=qf[:, 0:1], scalar2=None,
                op0=Alu.is_lt,
            )
            tq = pp.tile([P, Q], f32, tag="tq")
            nc.vector.tensor_mul(tq[:], Tall[:], qlt[:])
            carry4 = pp.tile([P, 1], f32, tag="carry4")
            nc.vector.tensor_reduce(
                out=carry4[:], in_=tq[:], axis=mybir.AxisListType.X, op=Alu.add
            )

            # ---------------- used-id limb bitmasks (from raw ids) --------
            # sh/limb/bit start from raw ids (no valid dependency); invalid
            # positions are knocked out by one late op on limb.
            sh = pp.tile([P, SQ], i32, tag="sh")
            nc.vector.tensor_single_scalar(
                out=sh[:], in_=ids_q[:], scalar=15, op=Alu.bitwise_and
            )
            limb = pp.tile([P, SQ], i32, tag="limb")
            nc.vector.tensor_single_scalar(
                out=limb[:], in_=ids_q[:], scalar=4, op=Alu.arith_shift_right
            )
            ones_i = pp.tile([P, SQ], i32, tag="ones_i")
            nc.vector.memset(ones_i[:], 1)
            bit = pp.tile([P, SQ], i32, tag="bit")
            nc.vector.tensor_tensor(
                out=bit[:], in0=ones_i[:], in1=sh[:], op=Alu.logical_shift_left
            )
            limb100 = pp.tile([P, SQ], i32, tag="limb100")
            nc.vector.tensor_single_scalar(
                out=limb100[:], in_=limb[:], scalar=100, op=Alu.add
            )
            # limb_m = valid ? limb : 100+limb  (never matches 0..7)
            limb_m = pp.tile([P, SQ], i32, tag="limb_m")
            nc.vector.scalar_tensor_tensor(
                out=limb_m[:], in0=valid[:], scalar=-100.0, in1=limb100[:],
                op0=Alu.mult, op1=Alu.add,
            )
            # all 8 limb masks in one [P, 8, SQ] tile, then a joint in-place
            # log-tree OR (int bitwise ops are DVE-only on this HW)
            M8 = pp.tile([P, 8, SQ], i32, tag="M8")
            for l in range(8):
                nc.vector.scalar_tensor_tensor(
                    out=M8[:, l, :], in0=limb_m[:], scalar=float(l), in1=bit[:],
                    op0=Alu.is_equal, op1=Alu.mult,
                )
            w = SQ // 2
            while w >= 1:
                nc.vector.tensor_tensor(
                    out=M8[:, :, 0:w], in0=M8[:, :, 0:w],
                    in1=M8[:, :, w : 2 * w], op=Alu.bitwise_or,
                )
                w //= 2
            limbs8 = pp.tile([P, 8], i32, tag="limbs8")
            nc.vector.tensor_copy(limbs8[:], M8[:, :, 0:1].rearrange("p l one -> p (l one)"))

            lm_dram = dramp.tile([P * 8], i32, tag="lm_dram")
            nc.sync.dma_start(out=lm_dram[:], in_=limbs8[:])
            # read back with every (b, q) partition seeing all 4 quarter sets
            limbs_q = pp.tile([P, 32], i32, tag="limbs_q")  # (q', l)
            nc.sync.dma_start(
                out=limbs_q[:],
                in_=lm_dram[:].rearrange("(b x) -> b x", x=32).unsqueeze(1)
                .broadcast_to([B, Q, 32]),
            )
            or1 = pp.tile([P, 16], i32, tag="or1")
            nc.vector.tensor_tensor(
                out=or1[:], in0=limbs_q[:, 0:16], in1=limbs_q[:, 16:32],
                op=Alu.bitwise_or,
            )
            limbs_f = pp.tile([P, 8], i32, tag="limbs_f")
            nc.vector.tensor_tensor(
                out=limbs_f[:], in0=or1[:, 0:8], in1=or1[:, 8:16], op=Alu.bitwise_or
            )

            # ---------------- expand limbs -> free0, ranks ----------------
            ub = pp.tile([P, N], i32, tag="ub")  # used bit per id
            nc.vector.tensor_tensor(
                out=ub[:].rearrange("b (l j) -> b l j", j=16),
                in0=limbs_f[:].unsqueeze(2).broadcast_to([P, 8, 16]),
                in1=iota16[:].rearrange("b (l j) -> b l j", j=16),
                op=Alu.logical_shift_right,
            )
            ub1 = pp.tile([P, N], i32, tag="ub1")
            nc.vector.tensor_single_scalar(
                out=ub1[:], in_=ub[:], scalar=1, op=Alu.bitwise_and
            )
            free0 = pp.tile([P, N], f32, tag="free0")  # 1 - (ub & 1)
            nc.vector.tensor_scalar(
                out=free0[:], in0=ub1[:], scalar1=-1.0, scalar2=1.0,
                op0=Alu.mult, op1=Alu.add,
            )
            zerosN = pp.tile([P, N], f32, tag="zerosN")
            nc.vector.memset(zerosN[:], 0.0)
            rank_i = pp.tile([P, N], f32, tag="rank_i")
            nc.vector.tensor_tensor_scan(
                out=rank_i[:], data0=zerosN[:], data1=free0[:], initial=0.0,
                op0=Alu.add, op1=Alu.add,
            )
            rank_e = pp.tile([P, N], f32, tag="rank_e")
            nc.vector.tensor_sub(rank_e[:], rank_i[:], free0[:])
            r0 = pp.tile([P, 1], f32, tag="r0")  # 0 if id0 free else -10
            nc.vector.tensor_scalar(
                out=r0[:], in0=free0[:, 0:1], scalar1=1.0, scalar2=10.0,
                op0=Alu.subtract, op1=Alu.mult,
            )
            np10 = pp.tile([P, N], f32, tag="np10")
            nc.vector.tensor_copy(np10[:], iota_n[:])
            t1 = pp.tile([P, N], f32, tag="t1")
            nc.vector.tensor_add(t1[:], rank_e[:], np10[:])
            t2 = pp.tile([P, N], f32, tag="t2")
            nc.vector.tensor_mul(t2[:], t1[:], free0[:])
            rankp4 = pp.tile([P, N], f32, tag="rankp4")  # rank'
            nc.vector.tensor_sub(rankp4[:], t2[:], np10[:])
            nfr4 = pp.tile([P, 2], f32, tag="nfr4")  # [nfree, r0]
            nc.vector.tensor_copy(nfr4[:, 0:1], rank_i[:, N - 1 : N])
            nc.vector.tensor_copy(nfr4[:, 1:2], r0[:])

            # ---------------- k4 ----------------
            kexcl = pp.tile([P, SQ], f32, tag="kexcl")
            nc.vector.tensor_sub(kexcl[:], kincl[:], isnew[:])
            nc.vector.tensor_single_scalar(
                out=kexcl[:], in_=kexcl[:], scalar=carry4[:, 0:1], op=Alu.add
            )
            ovf = tmpp.tile([P, SQ], f32, tag="tmpq")
            nc.vector.tensor_single_scalar(
                out=ovf[:], in_=kexcl[:], scalar=nfr4[:, 0:1], op=Alu.is_ge
            )
            d1 = tmpp.tile([P, SQ], f32, tag="tmpq")
            nc.vector.tensor_single_scalar(
                out=d1[:], in_=kexcl[:], scalar=nfr4[:, 1:2], op=Alu.subtract
            )
            e1 = tmpp.tile([P, SQ], f32, tag="tmpq")
            nc.vector.tensor_mul(e1[:], ovf[:], d1[:])
            k4a = tmpp.tile([P, SQ], f32, tag="tmpq")
            nc.vector.tensor_sub(k4a[:], kexcl[:], e1[:])
            k4b = tmpp.tile([P, SQ], f32, tag="tmpq")
            nc.vector.scalar_tensor_tensor(
                out=k4b[:], in0=k4a[:], scalar=2.0, in1=isnew[:],
                op0=Alu.add, op1=Alu.mult,
            )
            k4 = pp.tile([P, SQ], f32, tag="k4")
            nc.vector.tensor_single_scalar(
                out=k4[:], in_=k4b[:], scalar=-2.0, op=Alu.add
            )

            # ---------------- output equality grid + store ----------------
            skip_eq = bool(os.environ.get("K_PROBE_NO_EQ"))
            skip_odma = bool(os.environ.get("K_PROBE_NO_ODMA"))
            out_v = out_d[:].rearrange("b (q x) n -> (b q) x n", q=Q)
            n_gp = int(os.environ.get("K_EQ_GPSIMD", "0"))
            # ramped block sizes: tiny first blocks let the store stream start
            # as soon as rankp4/k4 are ready; steady-state blocks are 32 wide.
            sizes = [4, 4, 8, 16] + [32] * 15
            assert sum(sizes) == SQ
            off = 0
            for blk, bw in enumerate(sizes):
                osb = outp.tile([P, 32, N], f32, tag="osb")
                on_pool = blk >= len(sizes) - n_gp
                if not skip_eq:
                    in0 = rankp4[:].unsqueeze(1).broadcast_to([P, bw, N])
                    in1 = (k4[:, off : off + bw].unsqueeze(2)
                           .broadcast_to([P, bw, N]))
                    if on_pool:
                        # Pool has no TensorScalarPtr opcode; use TensorTensor
                        nc.gpsimd.tensor_tensor(
                            out=osb[:, 0:bw, :], in0=in0, in1=in1,
                            op=Alu.is_equal,
                        )
                    else:
                        nc.vector.scalar_tensor_tensor(
                            out=osb[:, 0:bw, :], in0=in0, scalar=0.0, in1=in1,
                            op0=Alu.add, op1=Alu.is_equal,
                        )
                if not skip_odma:
                    nc.sync.dma_start(
                        out=out_v[:, off : off + bw, :], in_=osb[:, 0:bw, :]
                    )
                off += bw

    nc.compile()
    return nc


_PROGRAM = None


def _get_program():
    global _PROGRAM
    if _PROGRAM is None:
        _PROGRAM = build_program()
    return _PROGRAM


def kernel(**inputs):
    from concourse import bass_utils

    ids = np.asarray(inputs["enref_ids"], dtype=np.int32)
    seq_len = np.asarray(inputs["enref_seq_len"], dtype=np.int32)
    logits = np.asarray(inputs["is_new_logits"], dtype=np.float32)
    assert ids.shape == (B_FULL, S), ids.shape
    assert seq_len.shape == (B_FULL,), seq_len.shape
    assert logits.shape == (B_FULL, S, 2), logits.shape

    nc = _get_program()
    in_maps = []
    for c in range(N_CORES):
        sl = slice(c * B, (c + 1) * B)
        in_maps.append(
            {
                "enref_ids": np.ascontiguousarray(ids[sl]),
                "enref_seq_len": np.ascontiguousarray(seq_len[sl]),
                "is_new_logits": np.ascontiguousarray(logits[sl]),
            }
        )
    res = bass_utils.run_bass_kernel_spmd(nc, in_maps, list(range(N_CORES)))
    global _LAST_RESULTS
    _LAST_RESULTS = res
    out = np.concatenate([res.results[i]["out"] for i in range(N_CORES)], axis=0)
    return out.astype(np.float32, copy=False)


_LAST_RESULTS = None

